# revision 43
# baseline (speedup 1.0000x reference)
"""Multi-head attention (per-head projections + relative position bias) on 8
Trainium2 NeuronCores.

Sharding: core c -> batch c//4, heads 4*(c%4) .. 4*(c%4)+4 (tensor parallel
over heads within a batch). Each core computes its 4 heads end-to-end plus the
partial output projection for those heads; the host sums the 4 partials per
batch and adds bfc.

v2 design notes (vs the v1 baseline at 512us):
- The bias add is folded multiplicatively: host ships eb = exp(bias/8) and the
  device computes esb = exp(qk/8) * eb. This removes the per-tile f32 PSUM
  tensor_tensor adds from DVE (185us at 1x mode) and lets ACT read scores
  straight from PSUM.
- Softmax denominators use reciprocal_approx_fast (single custom-DVE op)
  instead of InstReciprocal (iterative divide, 6.5us per [1,1024] tile), and
  the per-q broadcast runs on the idle GPSIMD engine (partition_broadcast)
  instead of a PE matmul into PSUM, freeing PSUM banks.
- Scores accumulate into [128,1024] f32 PSUM tiles (2 banks, double buffered)
  so exp runs as N=1024 ACT ops; PSUM: 4 banks scores + 4 banks AV accum.
- FC packs head pairs along 128 partitions (K=128 instead of 2x K=64) and the
  output is written/DMA'd as f16.
- eb is pre-tiled contiguously on the host so each (group, tt) bias tile is a
  single 512KB DMA.
"""

import sys

sys.path.insert(0, "/opt/trn_rl_repo")

import numpy as np

import concourse.bass as bass
import concourse.tile as tile_mod
from concourse import bass_utils as _bass_utils
from concourse import library_config, mybir

# ---------------------------------------------------------------------------
# Enable walrus's LDWEIGHTS-dedup pass: back-to-back matmuls reusing the same
# stationary operand skip the redundant weight reload. The AV matmuls (K=128,
# same vv block for both q-chunks) otherwise serialize LDW+MM at ~425ns each.
# ---------------------------------------------------------------------------
# (walrus rejects --enable-ldw-opt=true with pre-split InstLdweights)

# ---------------------------------------------------------------------------
# This walrus build accepts only one sem-wait per CTRL/Drain instruction, so
# split the TileContext tail drain's waits onto individual single-wait nops.
# ---------------------------------------------------------------------------


def _patched_drain_and_barrier(self, tick_clock, wait_clock):
    nc = self.nc
    drain_inst = nc.sync.drain()
    wait_clock.add_sem_waits(
        drain_inst.ins, tile_mod.ScopedClock({None: tick_clock.global_clock})
    )
    si = drain_inst.ins.sync_info
    if si is not None and si.on_wait is not None and len(si.on_wait) > 1:
        waits = list(si.on_wait)
        si.on_wait = [waits[0]]
        for w in waits[1:]:
            n = nc.sync.nop()
            n.ins.sync_info = mybir.SyncInfo(on_wait=[w], on_update=[])

    nc.all_engine_barrier()
    assert self.sems is not None
    popped = nc._tile_sem_poison_stack.pop()
    assert popped is self._sem_poison
    nc.clear_and_free_semaphores(list(self.sems.allocated().values()))
    nc.all_engine_barrier()


tile_mod.TileContext._drain_and_barrier = _patched_drain_and_barrier

_split_ctr = [0]


def _split_multi_waits(nc):
    """Walrus here accepts a single sem-wait per instruction; hoist extra waits
    onto single-wait nops inserted just before, on the same engine."""
    for f in nc.m.functions:
        for bb in f.blocks:
            insts = bb.instructions
            out = []
            for inst in insts:
                si = inst.sync_info
                if si is not None and si.on_wait is not None and len(si.on_wait) > 1:
                    waits = list(si.on_wait)
                    for w in waits[:-1]:
                        _split_ctr[0] += 1
                        n = mybir.InstNoOp(name=f"splitw-{_split_ctr[0]}", ins=[], outs=[])
                        n.engine = inst.engine
                        n.sync_info = mybir.SyncInfo(on_wait=[w], on_update=[])
                        out.append(n)
                    inst.sync_info = mybir.SyncInfo(
                        on_wait=[waits[-1]], on_update=list(si.on_update or [])
                    )
                out.append(inst)
            if len(out) != len(insts):
                bb.instructions[:] = out

def _verify_ldw_dedup(nc):
    """Every matmul with ldweights=False must immediately follow a matmul
    with identical stationary operand in the final tensor-engine order."""
    for f in nc.m.functions:
        for bb in f.blocks:
            prev_w = None
            for inst in bb.instructions:
                if isinstance(inst, mybir.InstMatmult):
                    w = str(inst.ins[1])
                    if inst.ldweights is False:
                        assert prev_w == w, (
                            f"ldweights=False matmul {inst.name} not preceded "
                            f"by same-weights matmul:\n{prev_w}\nvs\n{w}"
                        )
                    prev_w = w


B, S, D, H, DH = 2, 2048, 1024, 16, 64
NCORES = 8
HPC = 4  # heads per core
P = 128
F16 = mybir.dt.float16
F32 = mybir.dt.float32
AF = mybir.ActivationFunctionType
OP = mybir.AluOpType

GROUPS = ((0, 0), (1, 0), (0, 1), (1, 1))  # (pr, qh), qh-major

_cached = {}


def _build_program(split_waits=True):
    nc = bass.Bass("TRN2", target_bir_lowering=False, debug=False)

    # X tensors pre-arranged on host to the device layout [p, dp, s]
    d_xq = nc.dram_tensor("xq", [P, 8, S], F16, kind="ExternalInput").ap()
    d_xk = nc.dram_tensor("xk", [P, 8, S], F16, kind="ExternalInput").ap()
    d_xv = nc.dram_tensor("xv", [P, 8, S], F16, kind="ExternalInput").ap()
    # eb[pr, qh, tt, t(128), hh, q(1024)] = exp(bias/8) pre-tiled
    d_eb = nc.dram_tensor("eb", [2, 2, 16, P, 2, 1024], F16, kind="ExternalInput").ap()
    d_wq = nc.dram_tensor("wq", [P, 2, 8, P], F16, kind="ExternalInput").ap()
    d_wk = nc.dram_tensor("wk", [P, 2, 8, P], F16, kind="ExternalInput").ap()
    d_bq = nc.dram_tensor("bq", [2, P, 1], F32, kind="ExternalInput").ap()
    d_bk = nc.dram_tensor("bk", [2, P, 1], F32, kind="ExternalInput").ap()
    d_wv = nc.dram_tensor("wv", [P, 8, HPC * 65], F16, kind="ExternalInput").ap()
    d_bv = nc.dram_tensor("bv", [1, HPC * 65], F16, kind="ExternalInput").ap()
    # wfc[j(128 = pair heads stacked), pair, e(1024)]
    d_wfc = nc.dram_tensor("wfc", [P, 2, D], F16, kind="ExternalInput").ap()
    d_out = nc.dram_tensor("out", [S, D], F16, kind="ExternalOutput").ap()
    # scratch for softmax-denominator reciprocal rows (broadcast roundtrip)
    d_rec = nc.dram_tensor("recscr", [2, 4, 1024], F16, kind="Internal").ap()

    with tile_mod.TileContext(nc) as tc:
        with tc.tile_pool(name="persist", bufs=1) as persist, \
             tc.tile_pool(name="ebpool", bufs=8) as ebpool:
            qT = persist.tile([P, 2, S], F16, tag="qT")  # [j(2 heads), pair, s]
            kT = persist.tile([P, 2, S], F16, tag="kT")
            # [t_in, t_tile, h*128+j]: 65 data cols per head (64 v + ones),
            # zero-padded to 128 so AV weight loads qualify for FWL
            vv = persist.tile([P, 16, HPC * P], F16, tag="vv")
            # onorm2[j2(128 = pair stacked), pair, q]
            onorm2 = persist.tile([P, 2, S], F16, tag="onorm2")
            wfc_sb = persist.tile([P, 2, D], F16, tag="wfc")
            bq_sb = persist.tile([P, 2], F32, tag="bq")
            bk_sb = persist.tile([P, 2], F32, tag="bk")
            ones_sb = persist.tile([1, P], F16, tag="ones")
            bv_sb = persist.tile([1, HPC * 65], F16, tag="bv")

            nc.vector.memset(ones_sb[:], 1.0)
            nc.vector.memset(vv[:], 0.0)
            nc.sync.dma_start(bq_sb[:], d_bq.rearrange("pr p one -> p (pr one)"))
            nc.sync.dma_start(bk_sb[:], d_bk.rearrange("pr p one -> p (pr one)"))
            nc.sync.dma_start(bv_sb[:], d_bv[:])
            nc.sync.dma_start(wfc_sb[:], d_wfc[:])

            # ---------------- Phase A: projections ----------------
            with tc.tile_pool(name="xw", bufs=1) as xw, \
                 tc.tile_pool(name="ps_a", bufs=2, space="PSUM") as ps_a:
                # V first (warms the PE while xq/xk stream in), then Q/K.
                # X tensors arrive in dp-pair chunks so the accumulation loop
                # can start after the first chunk lands.
                wv_sb = xw.tile([P, 8, HPC * 65], F16, tag="wv")
                nc.sync.dma_start(wv_sb[:], d_wv[:])
                wq_sb = xw.tile([P, 2, 8, P], F16, tag="wq")
                nc.sync.dma_start(wq_sb[:], d_wq[:])
                wk_sb = xw.tile([P, 2, 8, P], F16, tag="wk")
                nc.sync.dma_start(wk_sb[:], d_wk[:])
                xq_sb = xw.tile([P, 8, S], F16, tag="xq")
                xk_sb = xw.tile([P, 8, S], F16, tag="xk")
                xv_sb = xw.tile([P, 8, S], F16, tag="xv")
                for dpc in range(4):
                    nc.sync.dma_start(
                        xv_sb[:, 2 * dpc : 2 * dpc + 2, :],
                        d_xv[:, 2 * dpc : 2 * dpc + 2, :],
                    )
                for dpc in range(4):
                    for x_sb, d_x in ((xq_sb, d_xq), (xk_sb, d_xk)):
                        nc.sync.dma_start(
                            x_sb[:, 2 * dpc : 2 * dpc + 2, :],
                            d_x[:, 2 * dpc : 2 * dpc + 2, :],
                        )

                # V (+ ones column via augmented bias row): natural [t, j]
                for tt in range(16):
                    psv = ps_a.tile([P, HPC * 65], F32, tag="psv", name=f"psv{tt}")
                    nc.tensor.matmul(
                        psv[:], lhsT=ones_sb[:, 0:P], rhs=bv_sb[:], start=True, stop=False
                    )
                    for dp in range(8):
                        nc.tensor.matmul(
                            psv[:],
                            lhsT=xv_sb[:, dp, tt * P : (tt + 1) * P],
                            rhs=wv_sb[:, dp, :],
                            start=False,
                            stop=(dp == 7),
                        )
                    nc.scalar.copy(
                        vv[:, tt, :].rearrange("p (h j) -> p h j", h=HPC)[:, :, 0:65],
                        psv[:],
                    )

                # Q^T / K^T: [2 heads stacked, s] per pair; + per-partition bias
                for sh in range(2):
                    for pr in range(2):
                        for w_sb, x_sb, b_sb, dst in (
                            (wq_sb, xq_sb, bq_sb, qT),
                            (wk_sb, xk_sb, bk_sb, kT),
                        ):
                            ps = ps_a.tile([P, 1024], F32, tag="ps_qk", name=f"ps{pr}{sh}")
                            for dp in range(8):
                                for qc in range(2):
                                    mm = nc.tensor.matmul(
                                        ps[:, qc * 512 : (qc + 1) * 512],
                                        lhsT=w_sb[:, pr, dp, :],
                                        rhs=x_sb[:, dp, sh * 1024 + qc * 512 : sh * 1024 + (qc + 1) * 512],
                                        start=(dp == 0),
                                        stop=(dp == 7),
                                    )
                            nc.scalar.activation(
                                dst[:, pr, sh * 1024 : (sh + 1) * 1024],
                                ps[:],
                                AF.Identity,
                                bias=b_sb[:, pr : pr + 1],
                                scale=1.0,
                            )

            # ---------------- Phase B: attention (+ interleaved FC) ----------
            with tc.tile_pool(name="espool", bufs=6) as espool, \
                 tc.tile_pool(name="esbpool", bufs=6) as esbpool, \
                 tc.tile_pool(name="otpool", bufs=6) as otpool, \
                 tc.tile_pool(name="recpool", bufs=2) as recpool, \
                 tc.tile_pool(name="rbpool", bufs=4) as rbpool, \
                 tc.tile_pool(name="fcpool", bufs=4) as fcpool, \
                 tc.tile_pool(name="ps_sc", bufs=2, space="PSUM") as ps_sc, \
                 tc.tile_pool(name="ps_av", bufs=2, space="PSUM") as ps_av:

                def emit_fc_one(qt):
                    # FC tile; psf rides the ps_s slot rotation briefly
                    if True:
                        psf = ps_sc.tile([P, 1024], F32, tag="ps_s", name=f"psf{qt}")
                        for p2 in range(2):
                            for ec in range(2):
                                mm = nc.tensor.matmul(
                                    psf[:, ec * 512 : (ec + 1) * 512],
                                    lhsT=onorm2[:, p2, qt * P : (qt + 1) * P],
                                    rhs=wfc_sb[:, p2, ec * 512 : (ec + 1) * 512],
                                    start=(p2 == 0),
                                    stop=(p2 == 1),
                                )
                        fo = fcpool.tile([P, D], F16, tag="fo", name=f"fo{qt}")
                        nc.scalar.copy(fo[:], psf[:])
                        eng = nc.sync if qt % 2 == 0 else nc.gpsimd
                        eng.dma_start(d_out[qt * P : (qt + 1) * P, :], fo[:])

                for pr, qh in GROUPS:
                    if pr == 0:
                        # rowsum rows for this qh land at partitions 0/32/64/96
                        rs_all = recpool.tile([97, 1024], F32, tag="rs", name=f"rs{qh}")
                        oT_qh = {}
                    po = [
                        ps_av.tile([P, 1024], F32, tag="ps_o", name=f"po{pr}{qh}{i}")
                        for i in range(2)
                    ]
                    for tt in range(16):
                        if (pr, qh) == (0, 1) and 4 <= tt < 12:
                            # spread qh0's FC through this group: each psf
                            # borrows a ps_s slot only briefly
                            emit_fc_one(tt - 4)
                        ebt = ebpool.tile([P, 2, 1024], F16, tag="eb", name=f"eb{pr}{qh}{tt}")
                        if (pr, qh) == GROUPS[0] and tt < 8:
                            # first prefetches ride the sync queue so they land
                            # behind the phase-A X loads, not before them
                            nc.sync.dma_start(ebt[:], d_eb[pr, qh, tt])
                        else:
                            nc.gpsimd.dma_start(ebt[:], d_eb[pr, qh, tt])
                        es = espool.tile([P, 2, 1024], F16, tag="es", name=f"es{pr}{qh}{tt}")
                        esb = esbpool.tile([P, 2, 1024], F16, tag="esb", name=f"esb{pr}{qh}{tt}")
                        for hh in range(2):
                            ps = ps_sc.tile([P, 1024], F32, tag="ps_s", name=f"ps{tt}{hh}")
                            for qc in range(2):
                                mm = nc.tensor.matmul(
                                    ps[:, qc * 512 : (qc + 1) * 512],
                                    lhsT=kT[hh * 64 : (hh + 1) * 64, pr, tt * P : (tt + 1) * P],
                                    rhs=qT[hh * 64 : (hh + 1) * 64, pr, qh * 1024 + qc * 512 : qh * 1024 + (qc + 1) * 512],
                                    start=True,
                                    stop=True,
                                )
                            nc.scalar.activation(es[:, hh, :], ps[:], AF.Exp, scale=0.125)
                        nc.vector.tensor_tensor(esb[:], es[:], ebt[:], OP.mult)
                        for hh in range(2):
                            h = 2 * pr + hh
                            for qc in range(2):
                                mm = nc.tensor.matmul(
                                    po[hh][:, qc * 512 : (qc + 1) * 512],
                                    lhsT=vv[:, tt, h * P : (h + 1) * P],
                                    rhs=esb[:, hh, qc * 512 : (qc + 1) * 512],
                                    start=(tt == 0),
                                    stop=(tt == 15),
                                )
                    # drain po to SBUF; gather rowsum rows at partition 32*(2pr+hh)
                    for hh in range(2):
                        oT = otpool.tile([64, 1024], F32, tag="oT", name=f"oT{pr}{qh}{hh}")
                        nc.vector.tensor_copy(oT[:], po[hh][0:64, :])
                        k4 = 2 * pr + hh
                        nc.vector.tensor_copy(
                            rs_all[32 * k4 : 32 * k4 + 1, :], po[hh][64:65, :]
                        )
                        oT_qh[(pr, hh)] = oT
                    if pr == 1:
                        # reciprocal as exp(-ln(rs)) on the scalar engine (ln
                        # and exp live in one table set); broadcast 1/rs along
                        # partitions via a DRAM roundtrip on the sync queue
                        # (same-queue -> ordered); no PSUM involved
                        lnrs = recpool.tile([97, 1024], F32, tag="lnrs", name=f"lnrs{qh}")
                        nc.scalar.activation(lnrs[:], rs_all[:], AF.Ln)
                        rec16 = recpool.tile([97, 1024], F16, tag="rec16", name=f"rec16{qh}")
                        nc.scalar.activation(rec16[:], lnrs[:], AF.Exp, scale=-1.0)
                        for k4 in range(4):
                            nc.sync.dma_start(
                                d_rec[qh, k4], rec16[32 * k4 : 32 * k4 + 1, :]
                            )
                        for ppr in range(2):
                            for hh in range(2):
                                k4 = 2 * ppr + hh
                                rb = rbpool.tile(
                                    [64, 1024], F16, tag="rb", name=f"rb{qh}{ppr}{hh}"
                                )
                                bsrc = bass.AP(
                                    d_rec.tensor,
                                    (qh * 4 + k4) * 1024,
                                    [[0, 64], [1, 1024]],
                                )
                                nc.sync.dma_start(rb[:, :], bsrc)
                                nc.vector.tensor_tensor(
                                    onorm2[hh * 64 : (hh + 1) * 64, ppr, qh * 1024 : (qh + 1) * 1024],
                                    oT_qh[(ppr, hh)][:],
                                    rb[:],
                                    OP.mult,
                                )
                        if qh == 1:
                            for qt in range(8, 16):
                                emit_fc_one(qt)

    if split_waits:
        _split_multi_waits(nc)
    return nc


def _prep_eb_all(relative_position_bias):
    """exp(bias/8) for the full tensor, f16, once."""
    return np.exp(
        0.125 * np.asarray(relative_position_bias, dtype=np.float32)
    ).astype(np.float16)


def _prep_core_inputs(c, query, key, value, eb_all, Wq, bq, Wk, bk, Wv, bv, Wfc):
    b = c // (NCORES // B)
    h0 = HPC * (c % (NCORES // B))
    f16 = np.float16

    # X: [D, S] transposed input -> device layout [p(128), dp(8), s]
    def xprep(x):
        xt = np.asarray(x.T, dtype=f16)  # [D, S]
        return np.ascontiguousarray(xt.reshape(8, P, S).transpose(1, 0, 2))

    xq = xprep(query[b])
    xk = xprep(key[b])
    xv = xprep(value[b])

    # eb tiles: [pr, qh, tt, t(128), hh, q(1024)]
    # eb_all[b, h] is [q, t]; device wants [t, q].
    y = eb_all[b, h0 : h0 + HPC]  # [4, q, t] f16
    yt = y.transpose(0, 2, 1)  # [4, t, q]
    eb = np.ascontiguousarray(
        yt.reshape(2, 2, 16, P, 2, 1024).transpose(0, 4, 2, 3, 1, 5)
    )  # [pr, qh, tt, t, hh, q]

    # wq/wk: per-pair [D, 128] -> [p, pr, dp, j]
    wq = np.stack(
        [np.concatenate([Wq[h0 + 2 * g], Wq[h0 + 2 * g + 1]], axis=1) for g in range(2)]
    ).astype(f16)  # [2, D, 128]
    wq = np.ascontiguousarray(wq.reshape(2, 8, P, P).transpose(2, 0, 1, 3))
    wk = np.stack(
        [np.concatenate([Wk[h0 + 2 * g], Wk[h0 + 2 * g + 1]], axis=1) for g in range(2)]
    ).astype(f16)
    wk = np.ascontiguousarray(wk.reshape(2, 8, P, P).transpose(2, 0, 1, 3))
    bqc = np.stack(
        [np.concatenate([bq[h0 + 2 * g], bq[h0 + 2 * g + 1]])[:, None] for g in range(2)]
    ).astype(np.float32)
    bkc = np.stack(
        [np.concatenate([bk[h0 + 2 * g], bk[h0 + 2 * g + 1]])[:, None] for g in range(2)]
    ).astype(np.float32)

    wv = np.zeros((D, HPC * 65), dtype=f16)
    bv_aug = np.zeros((1, HPC * 65), dtype=f16)
    for i in range(HPC):
        wv[:, i * 65 : i * 65 + 64] = Wv[h0 + i]
        bv_aug[0, i * 65 : i * 65 + 64] = bv[h0 + i]
        bv_aug[0, i * 65 + 64] = 1.0
    wv = np.ascontiguousarray(wv.reshape(8, P, HPC * 65).transpose(1, 0, 2))

    # wfc: [j(128 = pair heads stacked), pair, e]
    wfc = np.stack(
        [Wfc[(h0 + 2 * p) * DH : (h0 + 2 * p + 2) * DH] for p in range(2)]
    ).astype(f16)  # [2, 128, D]
    wfc = np.ascontiguousarray(wfc.transpose(1, 0, 2))

    return {
        "xq": xq, "xk": xk, "xv": xv, "eb": eb,
        "wq": wq, "wk": wk, "bq": bqc, "bk": bkc,
        "wv": wv, "bv": bv_aug, "wfc": wfc,
    }


def _install_ntff_hook():
    """The container's antenv stub lacks axon_hooks; synthesize it so
    trace=True can capture NTFF profiles via libaxon_pjrt.so ctypes calls."""
    import contextlib
    import ctypes
    import types

    import antenv

    if hasattr(antenv, "axon_hooks"):
        return
    so_path = "/opt/axon/libaxon_pjrt.so"
    try:
        lib = ctypes.CDLL(so_path)
    except OSError:
        return
    if not hasattr(lib, "axon_start_nrt_profile"):
        return
    lib.axon_start_nrt_profile.argtypes = [ctypes.POINTER(ctypes.c_int64), ctypes.c_size_t]
    lib.axon_start_nrt_profile.restype = ctypes.c_int64
    lib.axon_stop_nrt_profile.argtypes = [ctypes.c_char_p]
    lib.axon_stop_nrt_profile.restype = ctypes.c_int64

    @contextlib.contextmanager
    def _hook(output_dir, device_ids):
        import jax

        jax.devices()
        if device_ids:
            ids = (ctypes.c_int64 * len(device_ids))(*device_ids)
            rc = lib.axon_start_nrt_profile(ids, len(device_ids))
        else:
            rc = lib.axon_start_nrt_profile(None, 0)
        if rc != 0:
            raise RuntimeError(f"axon_start_nrt_profile rc={rc}")
        try:
            yield
        finally:
            n = lib.axon_stop_nrt_profile(str(output_dir).encode())
            print(f"profile: {n} file(s) written to {output_dir}", file=sys.stderr)

    mod = types.ModuleType("antenv.axon_hooks")
    mod._hook = _hook
    mod.get_axon_ntff_profile_hook = lambda: _hook
    mod.set_axon_ntff_profile_hook = lambda h: setattr(mod, "_hook", h)
    sys.modules["antenv.axon_hooks"] = mod
    antenv.axon_hooks = mod


def kernel(_trace=False, **inputs):
    from concourse.bass_utils import run_bass_kernel_spmd

    if _trace:
        _install_ntff_hook()
    if "nc" not in _cached:
        _cached["nc"] = _build_program()
    nc = _cached["nc"]

    args = {k: np.asarray(v) for k, v in inputs.items()}
    eb_all = _prep_eb_all(args["relative_position_bias"])
    in_maps = [
        _prep_core_inputs(
            c,
            args["query"], args["key"], args["value"],
            eb_all,
            args["Wq"], args["bq"], args["Wk"], args["bk"],
            args["Wv"], args["bv"], args["Wfc"],
        )
        for c in range(NCORES)
    ]

    res = run_bass_kernel_spmd(nc, in_maps, core_ids=list(range(NCORES)), trace=_trace)
    _cached["last_result"] = res

    out = np.zeros((B, S, D), dtype=np.float32)
    cpb = NCORES // B
    for b in range(B):
        out[b] = sum(
            res.results[b * cpb + i]["out"].astype(np.float32) for i in range(cpb)
        )
        out[b] += args["bfc"].astype(np.float32)[None, :]
    return out


# revision 44
# speedup vs baseline: 1.2152x; 1.2152x over previous
"""Multi-head attention (per-head projections + relative position bias) on 8
Trainium2 NeuronCores.

Sharding: core c -> batch c//4, heads 4*(c%4) .. 4*(c%4)+4 (tensor parallel
over heads within a batch). Each core computes its 4 heads end-to-end plus the
partial output projection for those heads; the host sums the 4 partials per
batch and adds bfc.

Design notes (vs the v1 baseline at 512us; this version measures ~317us):
- The bias add is folded multiplicatively: host ships eb = exp(bias/8) and the
  device computes esb = exp(qk/8) * eb. This removes the per-tile f32 PSUM
  tensor_tensor adds from DVE (185us at 1x mode) and lets ACT read scores
  straight from PSUM, one N=1024 exp per (tt, head).
- Softmax denominator reciprocal runs as exp(-ln(rs)) on the scalar engine
  (ln and exp share one ACT table set), batched over the 4 (pr, hh) rowsum
  rows per q-half at partitions 0/32/64/96; broadcast along partitions via
  ones outer-product matmuls into recycled AV-accumulator PSUM slots.
- All inputs are pre-arranged on the host into the exact device layout so
  every DMA is a large contiguous-per-partition transfer (the rearranged
  patterns had 256B lines and serialized the sync queue for ~40us).
- eb tiles stream on the gpsimd DMA queue (sync queue carries phase A).
- FC packs head pairs along 128 partitions (K=128 instead of 2x K=64), runs
  at the tail out of the scores-PSUM rotation slots, f16 output.
- PSUM: 4 banks double-buffered scores + 4 banks AV accumulators (2 heads,
  65 rows each: 64 output dims + a ones-column rowsum).
"""

import sys

sys.path.insert(0, "/opt/trn_rl_repo")

import numpy as np

import concourse.bass as bass
import concourse.tile as tile_mod
from concourse import mybir

# ---------------------------------------------------------------------------
# This walrus build accepts only one sem-wait per CTRL/Drain instruction, so
# split the TileContext tail drain's waits onto individual single-wait nops.
# ---------------------------------------------------------------------------


def _patched_drain_and_barrier(self, tick_clock, wait_clock):
    nc = self.nc
    drain_inst = nc.sync.drain()
    wait_clock.add_sem_waits(
        drain_inst.ins, tile_mod.ScopedClock({None: tick_clock.global_clock})
    )
    si = drain_inst.ins.sync_info
    if si is not None and si.on_wait is not None and len(si.on_wait) > 1:
        waits = list(si.on_wait)
        si.on_wait = [waits[0]]
        for w in waits[1:]:
            n = nc.sync.nop()
            n.ins.sync_info = mybir.SyncInfo(on_wait=[w], on_update=[])

    nc.all_engine_barrier()
    assert self.sems is not None
    popped = nc._tile_sem_poison_stack.pop()
    assert popped is self._sem_poison
    nc.clear_and_free_semaphores(list(self.sems.allocated().values()))
    nc.all_engine_barrier()


tile_mod.TileContext._drain_and_barrier = _patched_drain_and_barrier

_split_ctr = [0]


def _split_multi_waits(nc):
    """Walrus here accepts a single sem-wait per instruction; hoist extra waits
    onto single-wait nops inserted just before, on the same engine."""
    for f in nc.m.functions:
        for bb in f.blocks:
            insts = bb.instructions
            out = []
            for inst in insts:
                si = inst.sync_info
                if si is not None and si.on_wait is not None and len(si.on_wait) > 1:
                    waits = list(si.on_wait)
                    for w in waits[:-1]:
                        _split_ctr[0] += 1
                        n = mybir.InstNoOp(name=f"splitw-{_split_ctr[0]}", ins=[], outs=[])
                        n.engine = inst.engine
                        n.sync_info = mybir.SyncInfo(on_wait=[w], on_update=[])
                        out.append(n)
                    inst.sync_info = mybir.SyncInfo(
                        on_wait=[waits[-1]], on_update=list(si.on_update or [])
                    )
                out.append(inst)
            if len(out) != len(insts):
                bb.instructions[:] = out


B, S, D, H, DH = 2, 2048, 1024, 16, 64
NCORES = 8
HPC = 4  # heads per core
P = 128
F16 = mybir.dt.float16
F32 = mybir.dt.float32
AF = mybir.ActivationFunctionType
OP = mybir.AluOpType

GROUPS = ((0, 0), (1, 0), (0, 1), (1, 1))  # (pr, qh), qh-major

_cached = {}


def _build_program(split_waits=True):
    nc = bass.Bass("TRN2", target_bir_lowering=False, debug=False)

    # X tensors pre-arranged on host to the device layout [p, dp, s]
    d_xq = nc.dram_tensor("xq", [P, 8, S], F16, kind="ExternalInput").ap()
    d_xk = nc.dram_tensor("xk", [P, 8, S], F16, kind="ExternalInput").ap()
    d_xv = nc.dram_tensor("xv", [P, 8, S], F16, kind="ExternalInput").ap()
    # eb[pr, qh, tt, t(128), hh, q(1024)] = exp(bias/8) pre-tiled
    d_eb = nc.dram_tensor("eb", [2, 2, 16, P, 2, 1024], F16, kind="ExternalInput").ap()
    d_wq = nc.dram_tensor("wq", [P, 2, 8, P], F16, kind="ExternalInput").ap()
    d_wk = nc.dram_tensor("wk", [P, 2, 8, P], F16, kind="ExternalInput").ap()
    d_bq = nc.dram_tensor("bq", [2, P, 1], F32, kind="ExternalInput").ap()
    d_bk = nc.dram_tensor("bk", [2, P, 1], F32, kind="ExternalInput").ap()
    d_wv = nc.dram_tensor("wv", [P, 8, HPC * 65], F16, kind="ExternalInput").ap()
    d_bv = nc.dram_tensor("bv", [1, HPC * 65], F16, kind="ExternalInput").ap()
    # wfc[j(128 = pair heads stacked), pair, e(1024)]
    d_wfc = nc.dram_tensor("wfc", [P, 2, D], F16, kind="ExternalInput").ap()
    d_out = nc.dram_tensor("out", [S, D], F16, kind="ExternalOutput").ap()

    with tile_mod.TileContext(nc) as tc:
        with tc.tile_pool(name="persist", bufs=1) as persist, \
             tc.tile_pool(name="ebpool", bufs=8) as ebpool:
            qT = persist.tile([P, 2, S], F16, tag="qT")  # [j(2 heads), pair, s]
            kT = persist.tile([P, 2, S], F16, tag="kT")
            vv = persist.tile([P, 16, HPC * 65], F16, tag="vv")  # [t_in, t_tile, h*65+j]
            # onorm2[j2(128 = pair stacked), pair, q]
            onorm2 = persist.tile([P, 2, S], F16, tag="onorm2")
            wfc_sb = persist.tile([P, 2, D], F16, tag="wfc")
            bq_sb = persist.tile([P, 2], F32, tag="bq")
            bk_sb = persist.tile([P, 2], F32, tag="bk")
            ones_sb = persist.tile([1, P], F16, tag="ones")
            ones4 = persist.tile([97, 64], F16, tag="ones4")
            bv_sb = persist.tile([1, HPC * 65], F16, tag="bv")

            nc.vector.memset(ones_sb[:], 1.0)
            nc.vector.memset(ones4[:], 1.0)
            nc.sync.dma_start(bq_sb[:], d_bq.rearrange("pr p one -> p (pr one)"))
            nc.sync.dma_start(bk_sb[:], d_bk.rearrange("pr p one -> p (pr one)"))
            nc.sync.dma_start(bv_sb[:], d_bv[:])
            nc.sync.dma_start(wfc_sb[:], d_wfc[:])

            # ---------------- Phase A: projections ----------------
            with tc.tile_pool(name="xw", bufs=1) as xw, \
                 tc.tile_pool(name="ps_a", bufs=2, space="PSUM") as ps_a:
                # small weights first so the first matmuls aren't DMA-gated,
                # then X tensors in dp-pair chunks so the accumulation loop
                # can start after the first chunk lands
                wq_sb = xw.tile([P, 2, 8, P], F16, tag="wq")
                nc.sync.dma_start(wq_sb[:], d_wq[:])
                wk_sb = xw.tile([P, 2, 8, P], F16, tag="wk")
                nc.sync.dma_start(wk_sb[:], d_wk[:])
                wv_sb = xw.tile([P, 8, HPC * 65], F16, tag="wv")
                nc.sync.dma_start(wv_sb[:], d_wv[:])
                xq_sb = xw.tile([P, 8, S], F16, tag="xq")
                xk_sb = xw.tile([P, 8, S], F16, tag="xk")
                xv_sb = xw.tile([P, 8, S], F16, tag="xv")
                for dpc in range(4):
                    for x_sb, d_x in ((xq_sb, d_xq), (xk_sb, d_xk)):
                        nc.sync.dma_start(
                            x_sb[:, 2 * dpc : 2 * dpc + 2, :],
                            d_x[:, 2 * dpc : 2 * dpc + 2, :],
                        )
                for dpc in range(4):
                    nc.sync.dma_start(
                        xv_sb[:, 2 * dpc : 2 * dpc + 2, :],
                        d_xv[:, 2 * dpc : 2 * dpc + 2, :],
                    )

                # Q^T / K^T: [2 heads stacked, s] per pair; + per-partition bias
                for sh in range(2):
                    for pr in range(2):
                        for w_sb, x_sb, b_sb, dst in (
                            (wq_sb, xq_sb, bq_sb, qT),
                            (wk_sb, xk_sb, bk_sb, kT),
                        ):
                            ps = ps_a.tile([P, 1024], F32, tag="ps_qk", name=f"ps{pr}{sh}")
                            for dp in range(8):
                                for qc in range(2):
                                    nc.tensor.matmul(
                                        ps[:, qc * 512 : (qc + 1) * 512],
                                        lhsT=w_sb[:, pr, dp, :],
                                        rhs=x_sb[:, dp, sh * 1024 + qc * 512 : sh * 1024 + (qc + 1) * 512],
                                        start=(dp == 0),
                                        stop=(dp == 7),
                                    )
                            nc.scalar.activation(
                                dst[:, pr, sh * 1024 : (sh + 1) * 1024],
                                ps[:],
                                AF.Identity,
                                bias=b_sb[:, pr : pr + 1],
                                scale=1.0,
                            )

                # V (+ ones column via augmented bias row): natural [t, j]
                for tt in range(16):
                    psv = ps_a.tile([P, HPC * 65], F32, tag="psv", name=f"psv{tt}")
                    nc.tensor.matmul(
                        psv[:], lhsT=ones_sb[:, 0:P], rhs=bv_sb[:], start=True, stop=False
                    )
                    for dp in range(8):
                        nc.tensor.matmul(
                            psv[:],
                            lhsT=xv_sb[:, dp, tt * P : (tt + 1) * P],
                            rhs=wv_sb[:, dp, :],
                            start=False,
                            stop=(dp == 7),
                        )
                    nc.scalar.copy(vv[:, tt, :], psv[:])

            # ---------------- Phase B: attention (+ tail FC) ----------------
            with tc.tile_pool(name="espool", bufs=6) as espool, \
                 tc.tile_pool(name="esbpool", bufs=6) as esbpool, \
                 tc.tile_pool(name="otpool", bufs=6) as otpool, \
                 tc.tile_pool(name="recpool", bufs=2) as recpool, \
                 tc.tile_pool(name="fcpool", bufs=4) as fcpool, \
                 tc.tile_pool(name="ps_sc", bufs=2, space="PSUM") as ps_sc, \
                 tc.tile_pool(name="ps_av", bufs=2, space="PSUM") as ps_av:

                def emit_fc(qh):
                    # FC for this q-half; psf tiles ride the ps_s slot rotation
                    for qt in range(qh * 8, qh * 8 + 8):
                        psf = ps_sc.tile([P, 1024], F32, tag="ps_s", name=f"psf{qt}")
                        for p2 in range(2):
                            for ec in range(2):
                                nc.tensor.matmul(
                                    psf[:, ec * 512 : (ec + 1) * 512],
                                    lhsT=onorm2[:, p2, qt * P : (qt + 1) * P],
                                    rhs=wfc_sb[:, p2, ec * 512 : (ec + 1) * 512],
                                    start=(p2 == 0),
                                    stop=(p2 == 1),
                                )
                        fo = fcpool.tile([P, D], F16, tag="fo", name=f"fo{qt}")
                        nc.scalar.copy(fo[:], psf[:])
                        nc.gpsimd.dma_start(d_out[qt * P : (qt + 1) * P, :], fo[:])

                for pr, qh in GROUPS:
                    if pr == 0:
                        # rowsum rows for this qh land at partitions 0/32/64/96
                        rs_all = recpool.tile([97, 1024], F32, tag="rs", name=f"rs{qh}")
                        oT_qh = {}
                    po = [
                        ps_av.tile([65, 1024], F32, tag="ps_o", name=f"po{pr}{qh}{i}")
                        for i in range(2)
                    ]
                    for tt in range(16):
                        ebt = ebpool.tile([P, 2, 1024], F16, tag="eb", name=f"eb{pr}{qh}{tt}")
                        nc.gpsimd.dma_start(ebt[:], d_eb[pr, qh, tt])
                        es = espool.tile([P, 2, 1024], F16, tag="es", name=f"es{pr}{qh}{tt}")
                        esb = esbpool.tile([P, 2, 1024], F16, tag="esb", name=f"esb{pr}{qh}{tt}")
                        for hh in range(2):
                            ps = ps_sc.tile([P, 1024], F32, tag="ps_s", name=f"ps{tt}{hh}")
                            for qc in range(2):
                                nc.tensor.matmul(
                                    ps[:, qc * 512 : (qc + 1) * 512],
                                    lhsT=kT[hh * 64 : (hh + 1) * 64, pr, tt * P : (tt + 1) * P],
                                    rhs=qT[hh * 64 : (hh + 1) * 64, pr, qh * 1024 + qc * 512 : qh * 1024 + (qc + 1) * 512],
                                    start=True,
                                    stop=True,
                                )
                            nc.scalar.activation(es[:, hh, :], ps[:], AF.Exp, scale=0.125)
                        nc.vector.tensor_tensor(esb[:], es[:], ebt[:], OP.mult)
                        for hh in range(2):
                            h = 2 * pr + hh
                            for qc in range(2):
                                nc.tensor.matmul(
                                    po[hh][:, qc * 512 : (qc + 1) * 512],
                                    lhsT=vv[:, tt, h * 65 : (h + 1) * 65],
                                    rhs=esb[:, hh, qc * 512 : (qc + 1) * 512],
                                    start=(tt == 0),
                                    stop=(tt == 15),
                                )
                    # drain po to SBUF; gather rowsum rows at partition 32*(2pr+hh)
                    for hh in range(2):
                        oT = otpool.tile([64, 1024], F32, tag="oT", name=f"oT{pr}{qh}{hh}")
                        nc.vector.tensor_copy(oT[:], po[hh][0:64, :])
                        k4 = 2 * pr + hh
                        nc.vector.tensor_copy(
                            rs_all[32 * k4 : 32 * k4 + 1, :], po[hh][64:65, :]
                        )
                        oT_qh[(pr, hh)] = oT
                    if pr == 1:
                        if qh == 1:
                            # fill the final norm-chain window with qh0's FC
                            emit_fc(0)
                        # reciprocal as exp(-ln(rs)) on the scalar engine (ln and
                        # exp live in one table set); broadcast via ones
                        # outer-product matmuls into recycled po slots
                        lnrs = recpool.tile([97, 1024], F32, tag="lnrs", name=f"lnrs{qh}")
                        nc.scalar.activation(lnrs[:], rs_all[:], AF.Ln)
                        rec16 = recpool.tile([97, 1024], F16, tag="rec16", name=f"rec16{qh}")
                        nc.scalar.activation(rec16[:], lnrs[:], AF.Exp, scale=-1.0)
                        for ppr in range(2):
                            for hh in range(2):
                                k4 = 2 * ppr + hh
                                rb = ps_av.tile(
                                    [64, 1024], F32, tag="ps_o", name=f"rb{qh}{ppr}{hh}"
                                )
                                for qc in range(2):
                                    nc.tensor.matmul(
                                        rb[:, qc * 512 : (qc + 1) * 512],
                                        lhsT=ones4[32 * k4 : 32 * k4 + 1, :],
                                        rhs=rec16[32 * k4 : 32 * k4 + 1, qc * 512 : (qc + 1) * 512],
                                        start=True,
                                        stop=True,
                                        tile_position=(32 * k4, 0),
                                    )
                                nc.vector.tensor_tensor(
                                    onorm2[hh * 64 : (hh + 1) * 64, ppr, qh * 1024 : (qh + 1) * 1024],
                                    oT_qh[(ppr, hh)][:],
                                    rb[:],
                                    OP.mult,
                                )
                        if qh == 1:
                            emit_fc(1)

    if split_waits:
        _split_multi_waits(nc)
    return nc


def _prep_eb_all(relative_position_bias):
    """exp(bias/8) for the full tensor, f16, once."""
    return np.exp(
        0.125 * np.asarray(relative_position_bias, dtype=np.float32)
    ).astype(np.float16)


def _prep_core_inputs(c, query, key, value, eb_all, Wq, bq, Wk, bk, Wv, bv, Wfc):
    b = c // (NCORES // B)
    h0 = HPC * (c % (NCORES // B))
    f16 = np.float16

    # X: [D, S] transposed input -> device layout [p(128), dp(8), s]
    def xprep(x):
        xt = np.asarray(x.T, dtype=f16)  # [D, S]
        return np.ascontiguousarray(xt.reshape(8, P, S).transpose(1, 0, 2))

    xq = xprep(query[b])
    xk = xprep(key[b])
    xv = xprep(value[b])

    # eb tiles: [pr, qh, tt, t(128), hh, q(1024)]
    # eb_all[b, h] is [q, t]; device wants [t, q].
    y = eb_all[b, h0 : h0 + HPC]  # [4, q, t] f16
    yt = y.transpose(0, 2, 1)  # [4, t, q]
    eb = np.ascontiguousarray(
        yt.reshape(2, 2, 16, P, 2, 1024).transpose(0, 4, 2, 3, 1, 5)
    )  # [pr, qh, tt, t, hh, q]

    # wq/wk: per-pair [D, 128] -> [p, pr, dp, j]
    wq = np.stack(
        [np.concatenate([Wq[h0 + 2 * g], Wq[h0 + 2 * g + 1]], axis=1) for g in range(2)]
    ).astype(f16)  # [2, D, 128]
    wq = np.ascontiguousarray(wq.reshape(2, 8, P, P).transpose(2, 0, 1, 3))
    wk = np.stack(
        [np.concatenate([Wk[h0 + 2 * g], Wk[h0 + 2 * g + 1]], axis=1) for g in range(2)]
    ).astype(f16)
    wk = np.ascontiguousarray(wk.reshape(2, 8, P, P).transpose(2, 0, 1, 3))
    bqc = np.stack(
        [np.concatenate([bq[h0 + 2 * g], bq[h0 + 2 * g + 1]])[:, None] for g in range(2)]
    ).astype(np.float32)
    bkc = np.stack(
        [np.concatenate([bk[h0 + 2 * g], bk[h0 + 2 * g + 1]])[:, None] for g in range(2)]
    ).astype(np.float32)

    wv = np.zeros((D, HPC * 65), dtype=f16)
    bv_aug = np.zeros((1, HPC * 65), dtype=f16)
    for i in range(HPC):
        wv[:, i * 65 : i * 65 + 64] = Wv[h0 + i]
        bv_aug[0, i * 65 : i * 65 + 64] = bv[h0 + i]
        bv_aug[0, i * 65 + 64] = 1.0
    wv = np.ascontiguousarray(wv.reshape(8, P, HPC * 65).transpose(1, 0, 2))

    # wfc: [j(128 = pair heads stacked), pair, e]
    wfc = np.stack(
        [Wfc[(h0 + 2 * p) * DH : (h0 + 2 * p + 2) * DH] for p in range(2)]
    ).astype(f16)  # [2, 128, D]
    wfc = np.ascontiguousarray(wfc.transpose(1, 0, 2))

    return {
        "xq": xq, "xk": xk, "xv": xv, "eb": eb,
        "wq": wq, "wk": wk, "bq": bqc, "bk": bkc,
        "wv": wv, "bv": bv_aug, "wfc": wfc,
    }


def _install_ntff_hook():
    """The container's antenv stub lacks axon_hooks; synthesize it so
    trace=True can capture NTFF profiles via libaxon_pjrt.so ctypes calls."""
    import contextlib
    import ctypes
    import types

    import antenv

    if hasattr(antenv, "axon_hooks"):
        return
    so_path = "/opt/axon/libaxon_pjrt.so"
    try:
        lib = ctypes.CDLL(so_path)
    except OSError:
        return
    if not hasattr(lib, "axon_start_nrt_profile"):
        return
    lib.axon_start_nrt_profile.argtypes = [ctypes.POINTER(ctypes.c_int64), ctypes.c_size_t]
    lib.axon_start_nrt_profile.restype = ctypes.c_int64
    lib.axon_stop_nrt_profile.argtypes = [ctypes.c_char_p]
    lib.axon_stop_nrt_profile.restype = ctypes.c_int64

    @contextlib.contextmanager
    def _hook(output_dir, device_ids):
        import jax

        jax.devices()
        if device_ids:
            ids = (ctypes.c_int64 * len(device_ids))(*device_ids)
            rc = lib.axon_start_nrt_profile(ids, len(device_ids))
        else:
            rc = lib.axon_start_nrt_profile(None, 0)
        if rc != 0:
            raise RuntimeError(f"axon_start_nrt_profile rc={rc}")
        try:
            yield
        finally:
            n = lib.axon_stop_nrt_profile(str(output_dir).encode())
            print(f"profile: {n} file(s) written to {output_dir}", file=sys.stderr)

    mod = types.ModuleType("antenv.axon_hooks")
    mod._hook = _hook
    mod.get_axon_ntff_profile_hook = lambda: _hook
    mod.set_axon_ntff_profile_hook = lambda h: setattr(mod, "_hook", h)
    sys.modules["antenv.axon_hooks"] = mod
    antenv.axon_hooks = mod


def kernel(_trace=False, **inputs):
    from concourse.bass_utils import run_bass_kernel_spmd

    if _trace:
        _install_ntff_hook()
    if "nc" not in _cached:
        _cached["nc"] = _build_program()
    nc = _cached["nc"]

    args = {k: np.asarray(v) for k, v in inputs.items()}
    eb_all = _prep_eb_all(args["relative_position_bias"])
    in_maps = [
        _prep_core_inputs(
            c,
            args["query"], args["key"], args["value"],
            eb_all,
            args["Wq"], args["bq"], args["Wk"], args["bk"],
            args["Wv"], args["bv"], args["Wfc"],
        )
        for c in range(NCORES)
    ]

    res = run_bass_kernel_spmd(nc, in_maps, core_ids=list(range(NCORES)), trace=_trace)
    _cached["last_result"] = res

    out = np.zeros((B, S, D), dtype=np.float32)
    cpb = NCORES // B
    for b in range(B):
        out[b] = sum(
            res.results[b * cpb + i]["out"].astype(np.float32) for i in range(cpb)
        )
        out[b] += args["bfc"].astype(np.float32)[None, :]
    return out
